# revision 53
# baseline (speedup 1.0000x reference)
"""Multi-head attention (B=4, S=2048, E=1024, H=16) on 8 trn2 NeuronCores.

Sharding: data-parallel over B (4) x tensor-parallel over H (2 halves of 8
heads). Core c handles batch c//2, head-half c%2. Column-parallel qkv_proj,
row-parallel out_proj; the all-reduce of the two partial outputs per batch is
done on the host during unshard (a sum of two arrays), as is the final
transpose (the device emits out^T to keep DMA writes contiguous).

Design (all matmuls bf16, fine-grained static interleave):
  - qkv proj in bf16 chunks of [128 dims x 512 tokens] (8 K-tiles each).
  - scores per (head, 512-q group, key-tile pair): psum [128 keys, 1024],
    exp on ACT -> e_all sbuf bf16. ACT (exp) is a ~266us near-co-bottleneck
    with PE (~281us), so qkv/out-proj chunks are interleaved as fillers to
    keep PE busy while ACT paces the attention stream.
  - PV flipped: stationary = e tile [128 keys, 128 q], moving = [v_h | 1]
    bf16 [128, 65] -> ctx psum [128 q, 65] accumulated over 16 key tiles;
    col 64 rides the softmax denominator.
  - normalize via DVE reciprocal + tensor_scalar_mul -> ctx bf16 [q, d];
    PE-transpose (identity matmul) -> ctxT [d, q]; v-bias added during the
    transpose evict (bias commutes with the softmax division).
  - out proj bf16: stationary wo [128 d, 128 e], moving ctxT [128 d, 512 q]
    -> out^T partial + bout (even cores only) -> DMA.
"""
import sys

import numpy as np
import ml_dtypes

sys.path.insert(0, "/opt/trn_rl_repo")

import concourse.bacc as bacc
import concourse.mybir as mybir
import concourse.tile as tile
from concourse.bass_utils import run_bass_kernel_spmd

F32 = mybir.dt.float32
BF16 = mybir.dt.bfloat16
EXP = mybir.ActivationFunctionType.Exp

B, S, E, H, HD = 4, 2048, 1024, 16, 64
HL = 8            # heads per core
SCALE = float(1.0 / np.sqrt(E))


def build_nc():
    nc = bacc.Bacc("TRN2", target_bir_lowering=False, debug=False, num_devices=8)
    xw_d = nc.declare_dram_parameter("xw", [E, 3584], BF16, isOutput=False)
    bqk_d = nc.declare_dram_parameter("bqk", [E, 1], F32, isOutput=False)
    bvb_d = nc.declare_dram_parameter("bvb", [128, 512], BF16, isOutput=False)
    bout_d = nc.declare_dram_parameter("bout", [E, 1], F32, isOutput=False)
    wo_d = nc.declare_dram_parameter("wo", [512, E], BF16, isOutput=False)
    out_d = nc.declare_dram_parameter("outT", [E, S], F32, isOutput=True)

    with tile.TileContext(nc) as tc:
      with tc.tile_pool(name="pp", bufs=1) as pp, \
           tc.tile_pool(name="ps", bufs=1, space="PSUM") as ps:
        # ---- persistent sbuf
        bqk_s = pp.tile([128, 8, 1], F32)
        bout_s = pp.tile([128, 8, 1], F32)
        bvb_s = pp.tile([128, 4, 128], BF16)
        wqk_s = pp.tile([128, 8, 1024], BF16)   # cols 0:512 q-dims, 512:1024 k
        wv_s = pp.tile([128, 8, 512], BF16)
        wo_s = pp.tile([128, 4, 1024], BF16)
        xc_s = [pp.tile([128, 8, 512], BF16, name=f"xc{g}") for g in range(4)]
        qk_s = pp.tile([128, 8, 2048], BF16)    # m 0-3 q pairs, 4-7 k pairs
        v1_s = pp.tile([128, 16, 8, 65], BF16)  # (jt, h, v|1)
        ctxT_s = pp.tile([128, 4, 2048], BF16)  # (d%128, d//128, q)
        warm = pp.tile([1, 1], F32)

        # ---- prologue DMA, spread across Pool/SP/DVE queues ordered by need
        nc.gpsimd.dma_start(out=bqk_s, in_=bqk_d[:, :].rearrange(
            "(m p) o -> p m o", p=128))
        # wqk m-tiles on the Pool queue (k pair0, q pair0 first); xc3 and
        # wv follow pair 0 so the ACT queue carries no DMA at all (every
        # issue there delays the exp stream)
        for m in (4, 0, 5, 1, 6, 2, 7, 3):
            half, mi = (512, m - 4) if m >= 4 else (0, m)
            for kt in range(8):
                nc.gpsimd.dma_start(
                    out=wqk_s[:, kt, m * 128:(m + 1) * 128],
                    in_=xw_d[kt * 128:(kt + 1) * 128,
                             half + mi * 128:half + (mi + 1) * 128])
        # bout/bvb are not consumed until the first norms/outs (~25us in);
        # keep the early Pool issue slots for the critical weight tiles
        nc.gpsimd.dma_start(
            out=bvb_s, in_=bvb_d[:, :].rearrange("p (t c) -> p t c", c=128))
        nc.gpsimd.dma_start(out=bout_s, in_=bout_d[:, :].rearrange(
            "(m p) o -> p m o", p=128))
        for dt in range(4):
            nc.gpsimd.dma_start(out=wo_s[:, dt, :],
                                in_=wo_d[dt * 128:(dt + 1) * 128, :])
        for g in range(4):
            for kt in range(8):
                q = nc.sync if g < 3 else nc.scalar
                q.dma_start(out=xc_s[g][:, kt, :],
                            in_=xw_d[kt * 128:(kt + 1) * 128,
                                     1024 + g * 512:1024 + (g + 1) * 512])
            if g == 1:
                for kt in range(8):
                    nc.scalar.dma_start(
                        out=wv_s[:, kt, :],
                        in_=xw_d[kt * 128:(kt + 1) * 128, 3072:3584])
        nc.vector.memset(v1_s[:, :, :, 64:65], 1.0)
        nc.scalar.activation(out=warm, in_=bqk_s[0:1, 0, 0:1], func=EXP)

        # ---- chunk machinery: fillers sliced into ~430ns PE atoms so the
        # scores cadence (one [128,1024] pair per 1038ns of ACT exp) is never
        # blocked by a long accumulation run.
        chunk_state = {}   # id -> generator | "done"
        CHUNK_NS = {"kq": 1708.0, "v": 427.0, "out": 854.0}
        fil = {"ns": 0.0}               # un-emitted filler reservoir
        cur_block = [0]

        def kq_gen(m, gp):
            pq = ps.tile([128, 512], F32, tag="pfill", bufs=2,
                         name=f"pq_{m}_{gp}")
            for kt in range(8):
                nc.tensor.matmul(out=pq,
                                 lhsT=wqk_s[:, kt, m * 128:(m + 1) * 128],
                                 rhs=xc_s[gp][:, kt, :],
                                 start=(kt == 0), stop=(kt == 7))
                if kt % 2 == 1 and kt < 7:
                    yield 427.0
            nc.vector.tensor_scalar_add(
                qk_s[:, m, gp * 512:(gp + 1) * 512], pq, bqk_s[:, m, 0:1])
            yield 427.0

        def v_gen(jt, p):
            gp, off = jt // 4, (jt % 4) * 128
            pv = ps.tile([128, 128], F32, tag="pfill", bufs=2,
                         name=f"pvc_{jt}_{p}")
            for kt in range(8):
                nc.tensor.matmul(out=pv,
                                 lhsT=xc_s[gp][:, kt, off:off + 128],
                                 rhs=wv_s[:, kt, p * 128:(p + 1) * 128],
                                 start=(kt == 0), stop=(kt == 7))
            nc.vector.tensor_copy(
                v1_s[:, jt, 2 * p:2 * p + 2, 0:64],
                pv[:, :].rearrange("p (h d) -> p h d", d=64))
            yield 427.0

        def out_gen(g, et):
            po = ps.tile([128, 512], F32, tag="pfill", bufs=2,
                         name=f"po_{g}_{et}")
            for dt in range(4):
                nc.tensor.matmul(out=po,
                                 lhsT=wo_s[:, dt, et * 128:(et + 1) * 128],
                                 rhs=ctxT_s[:, dt, g * 512:(g + 1) * 512],
                                 start=(dt == 0), stop=(dt == 3))
                if dt == 1:
                    yield 427.0
            stg = pp.tile([128, 512], F32, tag="stg", bufs=4,
                          name=f"stg_{g}_{et}")
            nc.vector.tensor_scalar_add(stg, po, bout_s[:, et, 0:1])
            for hv in range(2):
                q = nc.gpsimd if hv == 0 else nc.sync
                q.dma_start(
                    out=out_d[et * 128:(et + 1) * 128,
                              g * 512 + hv * 256:g * 512 + (hv + 1) * 256],
                    in_=stg[:, hv * 256:(hv + 1) * 256])
            yield 427.0

        GENS = {"kq": kq_gen, "v": v_gen, "out": out_gen}
        F = []
        for p in range(4):
            F += [("v", jt, p) for jt in range(16)]
            if p < 3:
                F += [("kq", 4 + p + 1, gp) for gp in range(4)]
                F += [("kq", p + 1, g) for g in range(4)]
        fil["ns"] += sum(CHUNK_NS[c[0]] for c in F)
        # the kq chunks reached only via ensure() also sit in the reservoir
        fil["ns"] += 5 * CHUNK_NS["kq"]
        active = [None]   # (id, gen) of the single in-flight filler
        acc = {"pe": 0.0, "act": 0.0}   # emitted-work pacing counters

        def _gen_for(cid):
            st = chunk_state.get(cid)
            if st == "done":
                return None
            if st is None:
                st = GENS[cid[0]](*cid[1:])
                chunk_state[cid] = st
            return st

        def _drain(cid, gen):
            for c in gen:
                acc["pe"] += c
                fil["ns"] -= c
            chunk_state[cid] = "done"

        def ensure(cid):
            if chunk_state.get(cid) == "done":
                return
            if active[0] is not None:
                acid, agen = active[0]
                active[0] = None
                _drain(acid, agen)
                if acid == cid:
                    return
            gen = _gen_for(cid)
            if gen is not None:
                _drain(cid, gen)

        def pop_atoms():
            # keep emitted PE work paced to emitted ACT (exp) work so the
            # scores cadence never starves the activation engine; hold a
            # reserve for the out-proj gap around block 24
            while acc["pe"] < acc["act"]:
                if cur_block[0] < 23 and fil["ns"] <= 12000.0:
                    return
                if active[0] is None:
                    while F and chunk_state.get(F[0]) == "done":
                        F.pop(0)
                    if not F:
                        return
                    cid = F.pop(0)
                    active[0] = (cid, _gen_for(cid))
                cid, gen = active[0]
                try:
                    c = next(gen)
                    acc["pe"] += c
                    fil["ns"] -= c
                except StopIteration:
                    chunk_state[cid] = "done"
                    active[0] = None

        # ---- attention blocks: pairs-outer so each pair's k/v/q prep
        # spreads across the kernel instead of bunching in the first q-group
        LAG = 2
        blocks = [(g, 2 * p + hh)
                  for p in range(4) for g in range(4) for hh in range(2)]
        e_tiles = {}
        pctx_tiles = {}
        ctxn_tiles = {}

        def emit_scores(b, jp):
            g, h = blocks[b]
            p, r0 = h // 2, (h % 2) * 64
            ensure(("kq", 4 + p, jp // 2))
            ensure(("kq", p, g))
            pt = ps.tile([128, 1024], F32, tag="ps", bufs=2,
                         name=f"pt_{b}_{jp}")
            for i, jt in enumerate((2 * jp, 2 * jp + 1)):
                nc.tensor.matmul(
                    out=pt[:, i * 512:(i + 1) * 512],
                    lhsT=qk_s[r0:r0 + 64, 4 + p, jt * 128:(jt + 1) * 128],
                    rhs=qk_s[r0:r0 + 64, p, g * 512:(g + 1) * 512],
                    start=True, stop=True)
            nc.scalar.activation(out=e_tiles[b][:, jp, :], in_=pt, func=EXP,
                                 scale=SCALE)

        def pv_half(b, qt, half):
            g, h = blocks[b]
            if qt == 0 and half == 0 and b not in pctx_tiles:
                pctx_tiles[b] = ps.tile([128, 4, 65], F32, tag="pctx",
                                        bufs=2, name=f"pctx_{b}")
            e_t, pc = e_tiles[b], pctx_tiles[b]
            if qt == 0:
                for jt in range(half * 8, half * 8 + 8):
                    ensure(("v", jt, h // 2))
            for jt in range(half * 8, half * 8 + 8):
                off = (jt % 2) * 512 + qt * 128
                nc.tensor.matmul(out=pc[:, qt, :],
                                 lhsT=e_t[:, jt // 2, off:off + 128],
                                 rhs=v1_s[:, jt, h, :],
                                 start=(jt == 0), stop=(jt == 15))
            return 217.0

        def norm(b, qt):
            """Normalize one head's ctx into its half of the pair tile; the
            v-bias rides along ((ctx/den) + bv via scalar_tensor_tensor)."""
            g, h = blocks[b]
            hh, dt = h % 2, h // 2
            pc = pctx_tiles[b]
            rr = pp.tile([128, 1], F32, tag="rrec", bufs=3,
                         name=f"rr_{b}_{qt}")
            nc.vector.reciprocal(rr, pc[:, qt, 64:65])
            key = (b // 2, qt)
            if key not in ctxn_tiles:
                ctxn_tiles[key] = pp.tile([128, 128], BF16, tag="ctxn",
                                          bufs=6, name=f"cn_{b // 2}_{qt}")
            nc.vector.scalar_tensor_tensor(
                out=ctxn_tiles[key][:, hh * 64:(hh + 1) * 64],
                in0=pc[:, qt, 0:64], scalar=rr[:, 0:1],
                in1=bvb_s[:, dt, hh * 64:(hh + 1) * 64],
                op0=mybir.AluOpType.mult, op1=mybir.AluOpType.add)
            return 0.0

        def transpose_pair(b, qt):
            """[128 q, 128 d] -> ctxT [128 d, 128 q] on the DMA xbar."""
            g, h = blocks[b]
            dt = h // 2
            nc.sync.dma_start_transpose(
                ctxT_s[:, dt, g * 512 + qt * 128:g * 512 + (qt + 1) * 128],
                ctxn_tiles.pop((b // 2, qt))[:, :])
            return 0.0

        def pieces_for(b):
            g, h = blocks[b]
            ops = []
            for qt in range(4):
                ops.append(lambda b=b, qt=qt: pv_half(b, qt, 0))
                ops.append(lambda b=b, qt=qt: pv_half(b, qt, 1))
                if h % 2 == 1:
                    ops.append(lambda b=b, qt=qt: norm(b - 1, qt))
                    ops.append(lambda b=b, qt=qt: norm(b, qt))
                    ops.append(lambda b=b, qt=qt: transpose_pair(b, qt))
                    if h == 7 and qt == 3 and b != 31:
                        def add_outs(g=g):
                            for et in range(8):
                                F.append(("out", g, et))
                            fil["ns"] += 8 * CHUNK_NS["out"]
                            return 0.0
                        ops.append(add_outs)
            return ops

        for b in range(31):
            cur_block[0] = b
            e_tiles[b] = pp.tile([128, 8, 1024], BF16, tag="eall",
                                 bufs=LAG + 1, name=f"eall_{b}")
            pieces = pieces_for(b - LAG) if b >= LAG else []
            pi = 0
            for jp in range(8):
                emit_scores(b, jp)
                acc["pe"] += 426.0
                acc["act"] += 1038.0
                npc = ((jp + 1) * len(pieces)) // 8 - (jp * len(pieces)) // 8
                for _ in range(npc):
                    if pi < len(pieces):
                        acc["pe"] += pieces[pi]()
                        pi += 1
                pop_atoms()
            while pi < len(pieces):
                acc["pe"] += pieces[pi]()
                pi += 1
            e_tiles.pop(b - LAG - 1, None)
            pctx_tiles.pop(b - LAG - 1, None)

        acc["act"] += 8304.0      # budget one block of lookahead for pops
        while F and acc["pe"] < acc["act"]:
            pop_atoms()

        # ---- block 31, qt-major: scores for one 128-q column group at a
        # time, so chains/transpose/out-proj of early columns overlap the
        # last exps instead of trailing them
        cur_block[0] = 31
        e_tiles[31] = pp.tile([128, 8, 1024], BF16, tag="eall",
                              bufs=LAG + 1, name="eall_31")
        g31, h31 = blocks[31]
        p31, r31 = h31 // 2, (h31 % 2) * 64
        stg31 = pp.tile([128, 8, 512], F32)
        pieces = pieces_for(29)
        pi = 0

        def scores31(qt, half):
            pt = ps.tile([128, 1024], F32, tag="ps", bufs=2,
                         name=f"pt31_{qt}_{half}")
            for jj in range(8):
                jt = half * 8 + jj
                nc.tensor.matmul(
                    out=pt[:, jj * 128:(jj + 1) * 128],
                    lhsT=qk_s[r31:r31 + 64, 4 + p31, jt * 128:(jt + 1) * 128],
                    rhs=qk_s[r31:r31 + 64, p31,
                             g31 * 512 + qt * 128:g31 * 512 + (qt + 1) * 128],
                    start=True, stop=True)
            nc.scalar.activation(out=e_tiles[31][:, qt * 2 + half, :],
                                 in_=pt, func=EXP, scale=SCALE)
            acc["pe"] += 426.0
            acc["act"] += 1038.0

        def chain31(qt):
            if 31 not in pctx_tiles:
                pctx_tiles[31] = ps.tile([128, 4, 65], F32, tag="pctx",
                                         bufs=2, name="pctx_31")
            pc = pctx_tiles[31]
            for jt in range(16):
                nc.tensor.matmul(
                    out=pc[:, qt, :],
                    lhsT=e_tiles[31][:, qt * 2 + jt // 8,
                                     (jt % 8) * 128:(jt % 8 + 1) * 128],
                    rhs=v1_s[:, jt, h31, :],
                    start=(jt == 0), stop=(jt == 15))
            acc["pe"] += 433.0

        def chain30(qt):
            pv_half(30, qt, 0)
            pv_half(30, qt, 1)
            norm(30, qt)
            acc["pe"] += 433.0

        def outs31(qt):
            for et in range(8):
                po = ps.tile([128, 512], F32, tag="pfill", bufs=2,
                             name=f"po31_{et}_{qt}")
                for dt in range(4):
                    nc.tensor.matmul(
                        out=po[:, 0:128],
                        lhsT=wo_s[:, dt, et * 128:(et + 1) * 128],
                        rhs=ctxT_s[:, dt, g31 * 512 + qt * 128:
                                   g31 * 512 + (qt + 1) * 128],
                        start=(dt == 0), stop=(dt == 3))
                nc.vector.tensor_scalar_add(
                    stg31[:, et, qt * 128:(qt + 1) * 128], po[:, 0:128],
                    bout_s[:, et, 0:1])
                q = nc.gpsimd if et % 2 == 0 else nc.sync
                q.dma_start(
                    out=out_d[et * 128:(et + 1) * 128,
                              g31 * 512 + qt * 128:g31 * 512 + (qt + 1) * 128],
                    in_=stg31[:, et, qt * 128:(qt + 1) * 128])
            acc["pe"] += 1707.0

        # qt-pipelined sequence; all pieces(29) drain before the first
        # chain30 (its pctx-slot WAR needs block 28's norms complete)
        for qt in range(2):
            scores31(qt, 0)
            scores31(qt, 1)
            npc = (len(pieces) + 1 - qt) // (2 - qt)
            for _ in range(npc):
                if pi < len(pieces):
                    acc["pe"] += pieces[pi]()
                    pi += 1
            pop_atoms()
            if qt == 1:
                chain30(0)
                chain30(1)
        scores31(2, 0)
        scores31(2, 1)
        chain30(2)
        chain30(3)
        chain31(0)
        norm(31, 0)
        transpose_pair(31, 0)
        acc["act"] += 1e9
        while F or active[0] is not None:
            pop_atoms()
        chain31(1)
        norm(31, 1)
        transpose_pair(31, 1)
        scores31(3, 0)
        outs31(0)
        scores31(3, 1)
        chain31(2)
        norm(31, 2)
        transpose_pair(31, 2)
        outs31(1)
        chain31(3)
        norm(31, 3)
        transpose_pair(31, 3)
        outs31(2)
        outs31(3)


    nc.compile()
    return nc


_NC = None


def _get_nc():
    global _NC
    if _NC is None:
        _NC = build_nc()
    return _NC


def make_in_maps(query, Wqkv, bqkv, Wout, bout):
    query = np.asarray(query, dtype=np.float32)
    Wqkv = np.asarray(Wqkv, dtype=np.float32)
    bqkv = np.asarray(bqkv, dtype=np.float32)
    Wout = np.asarray(Wout, dtype=np.float32)
    bout = np.asarray(bout, dtype=np.float32)
    in_maps = []
    for c in range(8):
        b, hh = c // 2, c % 2
        heads = np.arange(hh * HL, hh * HL + HL)
        dims = (heads[:, None] * HD + np.arange(HD)[None, :]).reshape(-1)  # [512]
        q_rows, k_rows, v_rows = dims, E + dims, 2 * E + dims

        xw = np.empty((E, 3584), ml_dtypes.bfloat16)
        xw[:, 0:512] = Wqkv[q_rows].T.astype(ml_dtypes.bfloat16)
        xw[:, 512:1024] = Wqkv[k_rows].T.astype(ml_dtypes.bfloat16)
        xw[:, 1024:3072] = query[b].T.astype(ml_dtypes.bfloat16)
        xw[:, 3072:3584] = Wqkv[v_rows].T.astype(ml_dtypes.bfloat16)

        bqk = np.concatenate([bqkv[q_rows], bqkv[k_rows]]).reshape(E, 1)
        bvb = np.ascontiguousarray(np.broadcast_to(
            bqkv[v_rows], (128, 512))).astype(ml_dtypes.bfloat16)

        wo = np.ascontiguousarray(Wout[:, dims].T).astype(ml_dtypes.bfloat16)
        bo = (bout if hh == 0 else np.zeros_like(bout)).reshape(E, 1)

        in_maps.append({
            "xw": xw, "bqk": np.ascontiguousarray(bqk), "bvb": bvb,
            "wo": wo, "bout": np.ascontiguousarray(bo),
        })
    return in_maps


def gather(results):
    out = np.empty((B, S, E), np.float32)
    for b in range(B):
        acc = results[2 * b]["outT"] + results[2 * b + 1]["outT"]   # [E, S]
        out[b] = acc.T
    return out


def kernel(query, key, value, Wqkv, bqkv, Wout, bout):
    # key/value are unused by the reference module (qkv all from query)
    nc = _get_nc()
    in_maps = make_in_maps(query, Wqkv, bqkv, Wout, bout)
    res = run_bass_kernel_spmd(nc, in_maps, list(range(8)))
    return gather(res.results)
